# revision 2
# baseline (speedup 1.0000x reference)
"""GroupedQueryAttention Trainium2 Bass kernel.

Sharding: 8 cores = (B=2) x (G=4 KV groups). Each core computes, for its
(batch b, kv-group g): the 4 query heads' Q/K/V projections, causal flash
attention, and a partial output projection Y^T_g = Wo[g-rows,:].T-contracted
with O^T. Host sums the 4 partials per batch and adds bo.

Everything on-chip is kept "transposed" (token dim T on the free axis):
  xT[d, t]   via PE transpose of x
  Q^T, K^T   directly from projection matmuls (W as stationary operand)
  S^T[s, t]  = (K^T s-block).T @ Q^T        (one 128-wide matmul per s-block)
  P^T        = exp(scale * S^T + mask)      (ACT, PSUM -> SBUF, bf16)
  O^T[dh, t] += (V s-block).T @ P^T         (PSUM accumulation over s-blocks)
  rowsum     += ones.T @ P^T                (PSUM accumulation, M=1)
  Y^T[dm, t] = sum_c (Wo chunk).T @ O^T_c   (per 128-row dm block)

Matmul operands are bf16 (1 cycle/row on the PE at any N); all accumulation
is fp32 in PSUM; softmax denominators are fp32 end-to-end.
"""

import sys

sys.path.insert(0, "/opt/trn_rl_repo")

from contextlib import ExitStack

import ml_dtypes
import numpy as np

import concourse.bass as bass  # noqa: F401  (import keeps bass registered)
import concourse.tile as tile
from concourse import bacc, mybir
from concourse.bass_utils import run_bass_kernel_spmd

F32 = mybir.dt.float32
BF16 = mybir.dt.bfloat16
AF = mybir.ActivationFunctionType

D = 2048          # model dim
T = 2048          # tokens
DH = 128          # head dim
G = 4             # kv groups
HPG = 4           # query heads per group
QC = HPG * DH     # query cols per group = 512
ND = D // 128     # 16 contraction chunks
NTAU = 4          # t tiles of 512
TW = 512          # t tile width
SCALE = DH ** -0.5
NEG = -1e30

TRACE = False
TRACE_KW = {}
LAST_RESULTS = None

_CACHE = {}


def _body(ctx, tc, xb, wq, wk, wv, wo, bq, bk, bv, masksd, identd, yT):
    nc = tc.nc

    # PSUM: acc(2) + st(3) + ot(2) + rs(1) = 8 banks
    psacc = ctx.enter_context(tc.tile_pool(name="psacc", bufs=2, space="PSUM"))
    psst = ctx.enter_context(tc.tile_pool(name="psst", bufs=3, space="PSUM"))
    psot = ctx.enter_context(tc.tile_pool(name="psot", bufs=2, space="PSUM"))
    psrs = ctx.enter_context(tc.tile_pool(name="psrs", bufs=1, space="PSUM"))

    consts = ctx.enter_context(tc.tile_pool(name="consts", bufs=1))
    ident = consts.tile([128, 128], BF16, tag="ident")
    nc.sync.dma_start(ident, identd)
    maskt = consts.tile([128, 4, TW], F32, tag="maskt")
    nc.sync.dma_start(maskt, masksd)
    bqt = consts.tile([128, 4], F32, tag="bqt")
    nc.sync.dma_start(bqt, bq.rearrange("(c p) -> p c", p=128))
    bkt = consts.tile([128, 1], F32, tag="bkt")
    nc.sync.dma_start(bkt, bk.rearrange("(c p) -> p c", p=128))
    bvt = consts.tile([128, 1], F32, tag="bvt")
    nc.sync.dma_start(bvt, bv.rearrange("(c p) -> p c", p=128))
    ones_col = consts.tile([128, 1], BF16, tag="ones_col")
    nc.vector.memset(ones_col, 1.0)

    # q/k/v activations survive until the flash loop
    qkv = ctx.enter_context(tc.tile_pool(name="qkv", bufs=1))
    qt = [qkv.tile([128, T], BF16, tag=f"qt{j}", name=f"qt{j}") for j in range(HPG)]
    kt = qkv.tile([128, T], BF16, tag="kt")
    vv = qkv.tile([128, ND, 128], BF16, tag="vv")  # [s%128, s_block, dh]

    # ---- phase A: load x (bf16), transpose on PE -> xT, and
    # ---- phase B: Q/K/V projections (contraction over d on partitions)
    with tc.tile_pool(name="xtp", bufs=ND) as xtp, \
         tc.tile_pool(name="xload", bufs=6) as xload, \
         tc.tile_pool(name="wstream", bufs=18) as wsp, \
         tc.tile_pool(name="vstage", bufs=2) as vts:

        xts = [xtp.tile([128, T], BF16, tag="xt", name=f"xt{d}") for d in range(ND)]
        for tg in range(4):  # groups of 4 token tiles of 128
            xtiles = []
            for i in range(4):
                it = tg * 4 + i
                xti = xload.tile([128, D], BF16, tag="x")
                nc.sync.dma_start(xti, xb[it * 128:(it + 1) * 128, :])
                xtiles.append(xti)
            for d in range(ND):
                pt = psst.tile([128, TW], BF16, tag="st")
                for i in range(4):
                    nc.tensor.transpose(
                        pt[:, i * 128:(i + 1) * 128],
                        xtiles[i][:, d * 128:(d + 1) * 128],
                        ident,
                    )
                dst = xts[d][:, tg * TW:(tg + 1) * TW]
                if d % 2 == 0:
                    nc.vector.tensor_copy(dst, pt)
                else:
                    nc.scalar.copy(dst, pt)

        # K projection: kt[:, sg] = (x @ Wk + bk)^T slice
        wkts = []
        for d in range(ND):
            wt = wsp.tile([128, DH], BF16, tag="wk")
            nc.sync.dma_start(wt, wk[d * 128:(d + 1) * 128, :])
            wkts.append(wt)
        for sg in range(NTAU):
            ps = psacc.tile([128, TW], F32, tag="acc")
            for d in range(ND):
                nc.tensor.matmul(ps, wkts[d], xts[d][:, sg * TW:(sg + 1) * TW],
                                 start=(d == 0), stop=(d == ND - 1))
            nc.scalar.activation(kt[:, sg * TW:(sg + 1) * TW], ps, AF.Identity,
                                 bias=bkt[:, 0:1])

        # V projection -> V^T staging tile -> PE transpose into native V
        wvts = []
        for d in range(ND):
            wt = wsp.tile([128, DH], BF16, tag="wv")
            nc.sync.dma_start(wt, wv[d * 128:(d + 1) * 128, :])
            wvts.append(wt)
        for sg in range(NTAU):
            ps = psacc.tile([128, TW], F32, tag="acc")
            for d in range(ND):
                nc.tensor.matmul(ps, wvts[d], xts[d][:, sg * TW:(sg + 1) * TW],
                                 start=(d == 0), stop=(d == ND - 1))
            vtt = vts.tile([128, TW], BF16, tag="vt")
            nc.scalar.activation(vtt, ps, AF.Identity, bias=bvt[:, 0:1])
            for i in range(4):
                sb = sg * 4 + i
                pv = psst.tile([128, 128], BF16, tag="st")
                nc.tensor.transpose(pv, vtt[:, i * 128:(i + 1) * 128], ident)
                nc.vector.tensor_copy(vv[:, sb, :], pv)

        # Q projection: full-width weight tiles, sliced per head block
        wqts = []
        for d in range(ND):
            wt = wsp.tile([128, QC], BF16, tag="wq")
            nc.sync.dma_start(wt, wq[d * 128:(d + 1) * 128, :])
            wqts.append(wt)
        for tau in range(NTAU):
            for cb in range(HPG):
                ps = psacc.tile([128, TW], F32, tag="acc")
                for d in range(ND):
                    nc.tensor.matmul(
                        ps, wqts[d][:, cb * 128:(cb + 1) * 128],
                        xts[d][:, tau * TW:(tau + 1) * TW],
                        start=(d == 0), stop=(d == ND - 1))
                nc.scalar.activation(qt[cb][:, tau * TW:(tau + 1) * TW], ps,
                                     AF.Identity, bias=bqt[:, cb:cb + 1])

    # ---- phase C: flash attention + phase D: output projection
    with tc.tile_pool(name="ptp", bufs=4) as ptp, \
         tc.tile_pool(name="norm", bufs=2) as nrm, \
         tc.tile_pool(name="otsb", bufs=1) as otp_pool, \
         tc.tile_pool(name="wop", bufs=1) as wop, \
         tc.tile_pool(name="ybounce", bufs=3) as yb:

        wot = [wop.tile([128, D], BF16, tag=f"wo{c}", name=f"wo{c}") for c in range(HPG)]
        for c in range(HPG):
            nc.sync.dma_start(wot[c], wo[c * 128:(c + 1) * 128, :])
        ots = [otp_pool.tile([128, T], BF16, tag=f"ot{j}", name=f"ots{j}") for j in range(HPG)]

        for tau in range(NTAU):
            nsb = 4 * tau + 4
            for j in range(HPG):
                otp = psot.tile([128, TW], F32, tag="ot")
                rs = psrs.tile([1, TW], F32, tag="rs")
                pts = {}
                qslice = qt[j][:, tau * TW:(tau + 1) * TW]

                def consume(sb, last):
                    # PV + rowsum matmuls for s-block sb
                    nc.tensor.matmul(otp, vv[:, sb, :], pts[sb],
                                     start=(sb == 0), stop=last)
                    nc.tensor.matmul(rs, ones_col, pts[sb],
                                     start=(sb == 0), stop=last)

                for sb in range(nsb):
                    st = psst.tile([128, TW], F32, tag="st")
                    nc.tensor.matmul(st, kt[:, sb * 128:(sb + 1) * 128], qslice,
                                     start=True, stop=True)
                    di = sb - 4 * tau
                    if di >= 0:  # diagonal block: causal mask
                        nc.vector.tensor_add(st, st, maskt[:, di, :])
                    pt = ptp.tile([128, TW], BF16, tag="pt")
                    nc.scalar.activation(pt, st, AF.Exp, scale=SCALE)
                    pts[sb] = pt
                    # software-pipeline PE: issue PV/rowsum 2 s-blocks behind
                    if sb >= 2:
                        consume(sb - 2, last=(sb - 2 == nsb - 1))
                        del pts[sb - 2]
                for sb in (nsb - 2, nsb - 1):
                    if sb >= 0 and sb in pts:
                        consume(sb, last=(sb == nsb - 1))

                # normalize: O^T / rowsum  (broadcast rowsum over partitions)
                rsb = nrm.tile([1, TW], F32, tag="rsb")
                nc.scalar.copy(rsb, rs)
                rc1 = nrm.tile([1, TW], F32, tag="rc1")
                nc.vector.reciprocal(rc1, rsb)
                rc128 = nrm.tile([128, TW], F32, tag="rc128")
                nc.gpsimd.partition_broadcast(rc128, rc1)
                nc.vector.tensor_mul(ots[j][:, tau * TW:(tau + 1) * TW],
                                     otp, rc128)

            # output projection for this tau: Y^T[dm, t] partial
            for m in range(ND):
                yp = psacc.tile([128, TW], F32, tag="acc")
                for c in range(HPG):
                    nc.tensor.matmul(
                        yp, wot[c][:, m * 128:(m + 1) * 128],
                        ots[c][:, tau * TW:(tau + 1) * TW],
                        start=(c == 0), stop=(c == HPG - 1))
                ys = yb.tile([128, TW], F32, tag="y")
                nc.scalar.copy(ys, yp)
                nc.sync.dma_start(
                    yT[m * 128:(m + 1) * 128, tau * TW:(tau + 1) * TW], ys)


def _build_nc():
    if "nc" in _CACHE:
        return _CACHE["nc"]
    nc = bacc.Bacc("TRN2", target_bir_lowering=False, debug=False)
    xb = nc.dram_tensor("xb", [T, D], BF16, kind="ExternalInput").ap()
    wq = nc.dram_tensor("wq", [D, QC], BF16, kind="ExternalInput").ap()
    wk = nc.dram_tensor("wk", [D, DH], BF16, kind="ExternalInput").ap()
    wv = nc.dram_tensor("wv", [D, DH], BF16, kind="ExternalInput").ap()
    wo = nc.dram_tensor("wo", [QC, D], BF16, kind="ExternalInput").ap()
    bq = nc.dram_tensor("bq", [QC], F32, kind="ExternalInput").ap()
    bk = nc.dram_tensor("bk", [DH], F32, kind="ExternalInput").ap()
    bv = nc.dram_tensor("bv", [DH], F32, kind="ExternalInput").ap()
    masksd = nc.dram_tensor("masks", [128, 4, TW], F32, kind="ExternalInput").ap()
    identd = nc.dram_tensor("ident", [128, 128], BF16, kind="ExternalInput").ap()
    yT = nc.dram_tensor("yT", [D, T], F32, kind="ExternalOutput").ap()

    with tile.TileContext(nc) as tc, ExitStack() as ctx:
        _body(ctx, tc, xb, wq, wk, wv, wo, bq, bk, bv, masksd, identd, yT)
    nc.compile()
    _CACHE["nc"] = nc
    return nc


def _host_consts():
    p = np.arange(128)[:, None, None]
    i = np.arange(4)[None, :, None]
    f = np.arange(TW)[None, None, :]
    masks = np.where(f >= i * 128 + p, 0.0, NEG).astype(np.float32)
    ident = np.eye(128, dtype=ml_dtypes.bfloat16)
    return masks, ident


def kernel(x, Wq, bq, Wk, bk, Wv, bv, Wo, bo):
    global LAST_RESULTS
    x = np.asarray(x, np.float32)
    Wq = np.asarray(Wq, np.float32)
    Wk = np.asarray(Wk, np.float32)
    Wv = np.asarray(Wv, np.float32)
    Wo = np.asarray(Wo, np.float32)
    bq = np.asarray(bq, np.float32)
    bk = np.asarray(bk, np.float32)
    bv = np.asarray(bv, np.float32)
    bo = np.asarray(bo, np.float32)

    nc = _build_nc()
    masks, ident = _host_consts()
    bf = lambda a: np.ascontiguousarray(a).astype(ml_dtypes.bfloat16)

    in_maps = []
    for c in range(8):
        b, g = divmod(c, G)
        in_maps.append({
            "xb": bf(x[b]),
            "wq": bf(Wq[:, g * QC:(g + 1) * QC]),
            "wk": bf(Wk[:, g * DH:(g + 1) * DH]),
            "wv": bf(Wv[:, g * DH:(g + 1) * DH]),
            "wo": bf(Wo[g * QC:(g + 1) * QC, :]),
            "bq": np.ascontiguousarray(bq[g * QC:(g + 1) * QC]),
            "bk": np.ascontiguousarray(bk[g * DH:(g + 1) * DH]),
            "bv": np.ascontiguousarray(bv[g * DH:(g + 1) * DH]),
            "masks": masks,
            "ident": ident,
        })

    res = run_bass_kernel_spmd(nc, in_maps, list(range(8)), trace=TRACE,
                               **TRACE_KW)
    LAST_RESULTS = res

    y = np.empty((2, T, D), np.float32)
    for b in range(2):
        acc = res.results[b * G + 0]["yT"].copy()
        for g in range(1, G):
            acc += res.results[b * G + g]["yT"]
        y[b] = acc.T + bo
    return y
